# revision 46
# baseline (speedup 1.0000x reference)
"""DHASPI level-loss kernel for 8 Trainium2 NeuronCores.

Data-parallel over the fused B*C row axis: each core gets 64 rows of x_env
(SBUF partitions 0-63) and 64 rows of y_env (partitions 64-127). The device
computes per-row energies of the 200 non-overlapping 960-sample blocks
(gcd(9600, 2880) — every overlapping loudness frame is a sum of 10 of
them); frame energies, gating, lufs, and the relu-diff loss are a tiny
float64 numpy epilogue on the host.

The 200 blocks per row are split across four routes chosen by a
linear-program balance of the four instruction queues (SP, Pool, ACT, DVE):

  route   load                 square + block reduction
  A  (84) Pool SWDGE cast->fp8 ACT Square with fused accum_out  (1172/blk)
  T8 (49) Pool SWDGE cast->fp8 DVE bn_stats per 480-chunk       (1120/blk)
  T32(24) SP HWDGE f32         DVE bn_stats per 480-chunk       (1120/blk)
  P  (43) SP HWDGE f32         Pool tensor_tensor square (bf16, 800/blk)
                               + Pool accum-DMA fold 960->480 (370/blk)
                               + DVE pair-add 480->240 + reduce_sum (385/blk)

Design notes:
- bn_stats (a real BIR opcode; the fused TENSOR_TENSOR_REDUCE ISA op does
  not codegen on this toolchain) computes per-480-chunk count/mean/M2 in
  f32; sum(x^2) = M2 + 240*mean_even^2 (+ odd half) is reassembled on the
  host from the "stats" output. Both ACT and bn_stats square in f32, so
  fp8 input costs only quantization noise whose squaring bias cancels
  between x and y.
- DMA completion semaphores fire per DMA-engine chunk, so a consumer that
  waits with zero margin can observe partition-banded pre-DMA data (the
  baseline was safe only via its scheduling slack). Defenses here: ACT
  waits one cast ahead (a tail dummy DMA supplies the final increments);
  the P-route fold is queued after the NEXT unit's square so the Q7 store
  pipe drains before the fold's DMA engines read it; the last P units
  wait one fold ahead (tail dummy fold); DVE-side consumers get a planned
  2us margin in the static schedule.
- Same-engine DVE RAW hazards (stale reads through the DVE write pipe):
  bn_stats reads only DMA-written tiles; the P-route reduce reads the
  pair-add output no faster than it was written.
- Engine instruction orders are static: consumer streams are built by a
  release-aware list scheduler iterated to a fixed point with producer
  timelines, and producers are deadline-sorted against consumer need
  times (margins included) with release clamping.
"""

import numpy as np

import concourse.bass as bass
from concourse import mybir
from concourse.bass_utils import run_bass_kernel_spmd

# Problem constants (hardcoded; kernel.py must be self-contained)
B, C, T = 16, 32, 192000
N_CORES = 8
ROWS = B * C  # 512
RPC = ROWS // N_CORES  # 64 rows per core per tensor

FRAME = 9600
SHIFT = 2880
BLK = 960  # gcd(FRAME, SHIFT)
NBLK = T // BLK  # 200 block sums per row
NFRM = (T - FRAME) // SHIFT + 1  # 64 frames per row

EPS = 1e-8
ALPHA = 1e-4
GAMMA_A = -70.0

F32 = mybir.dt.float32
BF16 = mybir.dt.bfloat16
FP8 = mybir.dt.float8e4

# Route unit sizes in 960-blocks (LP-balanced: A=83, T8=51, T32=26, P=40).
# Small first units cut pipeline fill.
A_SIZES = [1, 3] + [5] * 16  # 84 blocks -> ACT square+accum
T8_SIZES = [1, 3] + [5] * 9  # 49 blocks -> DVE bn_stats (fp8)
T32_SIZES = [2, 3, 4, 4, 4, 3, 3, 1]  # 24 blocks -> DVE bn_stats (f32)
P_SIZES = [1, 2] + [5] * 7 + [4, 1]  # 43 blocks -> Pool sq + fold + DVE add/red


def _spans(sizes, start):
    out = []
    for s in sizes:
        out.append((start, s))
        start += s
    return out, start


A_SPANS, _o = _spans(A_SIZES, 0)
T8_SPANS, _o = _spans(T8_SIZES, _o)
T32_SPANS, _o = _spans(T32_SIZES, _o)
P_SPANS, _o = _spans(P_SIZES, _o)
assert _o == NBLK

NABUF = 5  # fp8 ring for A route
NT8BUF = 5  # fp8 ring for T8 route
NT32BUF = 3  # f32 ring for T32 route
NPBUF = 4  # f32 ring for P route
NSQBUF = 3  # bf16 squared ring for P route

MAXU_A = max(A_SIZES) * BLK
MAXU_T8 = max(T8_SIZES) * BLK
MAXU_T32 = max(T32_SIZES) * BLK
MAXU_P = max(P_SIZES) * BLK

# ---------------------------------------------------------------------------
# Static scheduling: build one op list with dependencies and durations,
# list-schedule by latest-start-time, and use the per-engine order for
# emission.  Durations are V1-cost-model estimates in ns.

_DMA_LAT = 5500.0  # DMA init + completion-sem propagation + margin


def _dur(kind, nb):
    return {
        "cA": 370.6 * nb,
        "cT8": 370.6 * nb,
        "cT32": 1480.5 * nb,
        "cP": 1480.5 * nb,
        "sqP": 800.0 * nb,
        "f1P": max(370.6 * nb, 500.0),
        "actA": 1172.0 * nb,
        "ttr8": 1120.0 * nb,
        "ttr32": 1120.0 * nb,
        "redP": 400.0 * nb,
    }[kind]


def _schedule():
    """Three-pass schedule.

    1. Draft consumer streams (ACT sequential; DVE by target fractions),
       deadline-sort producers, and serially estimate SP/Pool timelines
       with release clamping.
    2. Rebuild the DVE stream by list-scheduling against estimated tile
       ready-times, so no DVE op sits in the stream before its data can
       exist.
    3. Re-sort producers against the final consumer need-times.
    """
    n8, n32, nP = len(T8_SPANS), len(T32_SPANS), len(P_SPANS)
    kind_spans = {"ttr8": T8_SPANS, "ttr32": T32_SPANS, "redP": P_SPANS}

    act_order = [("actA", k) for k in range(len(A_SPANS))]

    def act_need_of():
        need, t = {}, 0.0
        for kind, k in act_order:
            need[(kind, k)] = t
            t += _dur("actA", A_SPANS[k][1])
        return need

    act_need = act_need_of()

    def dve_need_of(order):
        need, t = {}, 0.0
        for kind, k in order:
            need[(kind, k)] = t
            t += _dur(kind, kind_spans[kind][k][1])
        return need, t

    # Pass 1 draft DVE order: fraction merge.
    units = []
    for k in range(n8):
        units.append(((k + 0.25) / n8, ("ttr8", k)))
    for k in range(n32):
        units.append(((k + 0.75) / n32, ("ttr32", k)))
    for k in range(nP):
        units.append(((k + 1.1) / (nP + 1.45), ("redP", k)))
    units.sort(key=lambda x: x[0])
    dve_order = [u for _, u in units]

    def producer_streams(dve_need, pool_done_prev=None):
        # consumer completion estimates (for ring-free releases)
        dve_done = {
            u: t + _dur(u[0], kind_spans[u[0]][u[1]][1])
            for u, t in dve_need.items()
        }
        act_done = {
            u: t + _dur("actA", A_SPANS[u[1]][1]) for u, t in act_need.items()
        }
        sem = 400.0  # engine-side sem propagation to the producer

        sp_ops = []
        for k in range(n32):
            rel = (
                dve_done[("ttr32", k - NT32BUF)] + sem if k >= NT32BUF else 0.0
            )
            dl = dve_need[("ttr32", k)] - _DMA_LAT
            sp_ops.append((dl, rel, ("cT32", k)))
        for k in range(nP):
            nb = P_SPANS[k][1]
            sq_dl = dve_need[("redP", k)] - _DMA_LAT - _dur("f1P", nb)
            rel = 0.0
            if k >= NPBUF and pool_done_prev is not None:
                rel = pool_done_prev.get(("sqP", k - NPBUF), 2900.0) + sem - 2600.0
            sp_ops.append((sq_dl - _DMA_LAT - _dur("sqP", nb), rel, ("cP", k)))
        sp_ops.sort(key=lambda x: x[0])
        t = 300.0
        sp_done = {}
        for _, rel, op in sp_ops:
            kind, k = op
            t = max(t, rel) + _dur(
                kind, (T32_SPANS if kind == "cT32" else P_SPANS)[k][1]
            )
            sp_done[op] = t + 2600.0
        pool_ops = []
        for k in range(len(A_SPANS)):
            rel = act_done[("actA", k - NABUF)] + sem if k >= NABUF else 0.0
            dl = act_need[("actA", max(k - 1, 0))] - _DMA_LAT
            pool_ops.append((dl, rel, ("cA", k)))
        for k in range(n8):
            rel = dve_done[("ttr8", k - NT8BUF)] + sem if k >= NT8BUF else 0.0
            pool_ops.append(
                (dve_need[("ttr8", k)] - _DMA_LAT - 1.0, rel, ("cT8", k))
            )
        for k in range(nP):
            nb = P_SPANS[k][1]
            f1_dl = dve_need[("redP", k)] - _DMA_LAT
            sq_dl = f1_dl - _dur("f1P", nb)
            rel = sp_done[("cP", k)]
            if k >= NSQBUF:
                rel = max(rel, dve_done[("redP", k - NSQBUF)] + sem)
            pool_ops.append((max(sq_dl, rel), rel, ("sqP", k)))
            pool_ops.append(
                (max(f1_dl, rel + _dur("sqP", nb)), rel, ("f1P", k))
            )
        pool_ops.sort(key=lambda x: x[0])
        t = 300.0
        pool_done = {}
        for _, rel, op in pool_ops:
            kind, k = op
            nb = (
                A_SPANS[k][1]
                if kind == "cA"
                else T8_SPANS[k][1]
                if kind == "cT8"
                else P_SPANS[k][1]
            )
            t = max(t, rel) + _dur(kind, nb)
            pool_done[op] = t + 2600.0
        return (
            [op for _, _, op in sp_ops],
            [op for _, _, op in pool_ops],
            sp_done,
            pool_done,
        )

    dve_need, dve_total = dve_need_of(dve_order)

    # Passes 2..N: alternate (a) producer timelines from current DVE order
    # and (b) DVE re-list-scheduled against tile ready-times, to a fixed
    # point; then a final producer re-sort.
    pool_done = None
    for it in range(4):
        if it < 3:
            sp_order, pool_order, sp_done, pool_done = producer_streams(
                dve_need, pool_done
            )
        # else: final pass — freeze producers, re-derive only the DVE order
        ready = {}
        for k in range(n8):
            ready[("ttr8", k)] = pool_done[("cT8", k)] + 2000.0
        for k in range(n32):
            ready[("ttr32", k)] = sp_done[("cT32", k)] + 2000.0
        for k in range(nP):
            ready[("redP", k)] = (
                pool_done[("f1P", k)] + 2000.0
                if k < nP - 3
                else pool_done[("f1P", min(k + 1, nP - 1))] + 1000.0
            )
        pend = set(ready)
        # within a kind, units must stay in k order: only the lowest
        # unscheduled k of each kind is eligible.
        ptr = {"ttr8": 0, "ttr32": 0, "redP": 0}
        now = 0.0
        new_order = []
        while pend:
            elig = [
                (kind, ptr[kind])
                for kind in ptr
                if (kind, ptr[kind]) in pend
            ]
            ready_now = [u for u in elig if ready[u] <= now]
            pick = (
                min(ready_now, key=lambda u: ready[u])
                if ready_now
                else min(elig, key=lambda u: ready[u])
            )
            now = max(now, ready[pick]) + _dur(
                pick[0], kind_spans[pick[0]][pick[1]][1]
            )
            new_order.append(pick)
            ptr[pick[0]] += 1
            pend.remove(pick)
        dve_order = new_order
        dve_need, dve_total = dve_need_of(dve_order)

    # f1P_k's emission waits on sqP_{min(k+1, nP-1)} (drain fence): enforce
    # that queue order or the Pool queue deadlocks on its own wait.
    reordered = []
    pending = {}  # required sq index -> [fold ops]
    seen_sq = -1
    for op in pool_order:
        kind, k = op
        if kind == "f1P":
            need = min(k + 1, nP - 1)
            if need > seen_sq:
                pending.setdefault(need, []).append(op)
                continue
            reordered.append(op)
        else:
            reordered.append(op)
            if kind == "sqP":
                seen_sq = k
                for j in sorted(list(pending)):
                    if j <= seen_sq:
                        reordered.extend(pending.pop(j))
    for j in sorted(list(pending)):
        reordered.extend(pending.pop(j))
    pool_order = reordered

    order = {
        "SP": sp_order,
        "Pool": pool_order,
        "ACT": act_order,
        "DVE": dve_order,
    }
    for eng, seq in order.items():
        by_kind = {}
        for i, (kind, k) in enumerate(seq):
            by_kind.setdefault(kind, []).append((i, k))
        for kind, pairs in by_kind.items():
            ks = sorted(k for _, k in pairs)
            for (i, _), k in zip(pairs, ks):
                seq[i] = (kind, k)
    return order, dve_total


_ORDER, _EST_FINISH = _schedule()


def _build_program() -> bass.Bass:
    nc = bass.Bass("TRN2", target_bir_lowering=False, debug=False)
    nc._op_labels = {}

    def tag(inst, label):
        nc._op_labels[inst.ins.name] = label
        return inst

    AF = mybir.ActivationFunctionType
    ALU = mybir.AluOpType
    AX = mybir.AxisListType

    xy = nc.dram_tensor("xy", [128, T], F32, kind="ExternalInput").ap()
    out = nc.dram_tensor("bs", [128, NBLK], F32, kind="ExternalOutput").ap()
    out_st = nc.dram_tensor(
        "stats", [128, (sum(T8_SIZES) + sum(T32_SIZES)) * 12], F32,
        kind="ExternalOutput",
    ).ap()

    abuf = [
        nc.alloc_sbuf_tensor(f"ab{i}", [128, MAXU_A], FP8).ap() for i in range(NABUF)
    ]
    t8buf = [
        nc.alloc_sbuf_tensor(f"t8b{i}", [128, MAXU_T8], FP8).ap()
        for i in range(NT8BUF)
    ]
    t32buf = [
        nc.alloc_sbuf_tensor(f"t32b{i}", [128, MAXU_T32], F32).ap()
        for i in range(NT32BUF)
    ]
    pbuf = [
        nc.alloc_sbuf_tensor(f"pb{i}", [128, MAXU_P], F32).ap() for i in range(NPBUF)
    ]
    sqbuf = [
        nc.alloc_sbuf_tensor(f"sq{i}", [128, MAXU_P], BF16).ap()
        for i in range(NSQBUF)
    ]
    bs = nc.alloc_sbuf_tensor("bst", [128, NBLK], F32).ap()
    junkA = nc.alloc_sbuf_tensor("junkA", [128, BLK], BF16).ap()
    nbn = sum(T8_SIZES) + sum(T32_SIZES)  # 73 bn blocks
    stats = nc.alloc_sbuf_tensor("stats_sb", [128, nbn * 12], F32).ap()
    BN_BASE = A_SPANS[-1][0] + A_SPANS[-1][1]  # first bn block index (84)

    with (
        nc.Block() as block,
        nc.semaphore("dmaA") as dmaA,
        nc.semaphore("dmaT8") as dmaT8,
        nc.semaphore("dmaT32") as dmaT32,
        nc.semaphore("dmaP") as dmaP,
        nc.semaphore("foldP") as foldP,
        nc.semaphore("aFree") as aFree,
        nc.semaphore("t8Free") as t8Free,
        nc.semaphore("t32Free") as t32Free,
        nc.semaphore("pf32Free") as pf32Free,
        nc.semaphore("sqFree") as sqFree,
        nc.semaphore("actfin") as actfin,
        nc.semaphore("dvefin") as dvefin,
        nc.semaphore("warm") as warm,
        nc.semaphore("outs") as outs,
    ):
        @block.sync
        def _(sync):
            for key in _ORDER["SP"]:
                kind, k = key
                if kind == "cT32":
                    if k >= NT32BUF:
                        sync.wait_ge(t32Free, k - NT32BUF + 1)
                    b0, nb = T32_SPANS[k]
                    tag(sync.dma_start(
                        out=t32buf[k % NT32BUF][:, 0 : nb * BLK],
                        in_=xy[:, b0 * BLK : (b0 + nb) * BLK],
                    ), ("cT32", k)).then_inc(dmaT32, 16)
                else:  # cP
                    if k >= NPBUF:
                        sync.wait_ge(pf32Free, k - NPBUF + 1)
                    b0, nb = P_SPANS[k]
                    tag(sync.dma_start(
                        out=pbuf[k % NPBUF][:, 0 : nb * BLK],
                        in_=xy[:, b0 * BLK : (b0 + nb) * BLK],
                    ), ("cP", k)).then_inc(dmaP, 16)
            nA = A_SPANS[-1][0] + A_SPANS[-1][1]
            p0 = P_SPANS[0][0]
            sync.wait_ge(actfin, 1)
            sync.dma_start(out=out[:, 0:nA], in_=bs[:, 0:nA]).then_inc(outs, 16)
            sync.wait_ge(dvefin, 1)
            sync.dma_start(
                out=out[:, p0:NBLK], in_=bs[:, p0:NBLK]
            ).then_inc(outs, 16)
            sync.dma_start(out=out_st, in_=stats).then_inc(outs, 16)
            sync.wait_ge(outs, 48)

        @block.gpsimd
        def _(g):
            n_cA = len(A_SPANS) - 1
            n_f1 = len(P_SPANS) - 1
            for key in _ORDER["Pool"]:
                kind, k = key
                if kind == "cA":
                    if k >= NABUF:
                        g.wait_ge(aFree, k - NABUF + 1)
                    b0, nb = A_SPANS[k]
                    tag(g.dma_start(
                        out=abuf[k % NABUF][:, 0 : nb * BLK],
                        in_=xy[:, b0 * BLK : (b0 + nb) * BLK],
                    ), ("cA", k)).then_inc(dmaA, 16)
                    if k == n_cA:
                        # tail dummy: its 16 chunk-completions certify every
                        # DMA engine has drained the last real cA transfer
                        g.dma_start(
                            out=junkA[:, 128:256], in_=xy[:, 0:128]
                        ).then_inc(dmaA, 16)
                elif kind == "cT8":
                    if k >= NT8BUF:
                        g.wait_ge(t8Free, k - NT8BUF + 1)
                    b0, nb = T8_SPANS[k]
                    tag(g.dma_start(
                        out=t8buf[k % NT8BUF][:, 0 : nb * BLK],
                        in_=xy[:, b0 * BLK : (b0 + nb) * BLK],
                    ), ("cT8", k)).then_inc(dmaT8, 16)
                elif kind == "sqP":
                    g.wait_ge(dmaP, 16 * (k + 1))
                    if k >= NSQBUF:
                        g.wait_ge(sqFree, k - NSQBUF + 1)
                    b0, nb = P_SPANS[k]
                    tag(g.tensor_tensor(
                        sqbuf[k % NSQBUF][:, 0 : nb * BLK],
                        pbuf[k % NPBUF][:, 0 : nb * BLK],
                        pbuf[k % NPBUF][:, 0 : nb * BLK],
                        op=ALU.mult,
                    ), ("sqP", k)).then_inc(pf32Free, 1)
                else:  # f1P: fold 960 -> 480, in place, accum add
                    g.wait_ge(pf32Free, min(k + 2, len(P_SPANS)))
                    b0, nb = P_SPANS[k]
                    v = (
                        sqbuf[k % NSQBUF][:, 0 : nb * BLK]
                        .rearrange("p (n b) -> p n b", b=BLK)
                    )
                    tag(g.dma_start(
                        out=v[:, :, 0:480], in_=v[:, :, 480:960], accum_op=ALU.add
                    ), ("f1P", k)).then_inc(foldP, 16)
                    if k == n_f1:
                        g.dma_start(
                            out=junkA[:, 256:320], in_=junkA[:, 320:384],
                            accum_op=ALU.add,
                        ).then_inc(foldP, 16)

        @block.scalar
        def _(scalar):
            # Warm the Square activation table during pipeline fill so the
            # 1383ns table load is off the first real unit's critical path.
            # (Input is a DVE-zeroed scratch; the output is never consumed.)
            scalar.wait_ge(warm, 1)
            scalar.activation(junkA[:, 8:16], junkA[:, 0:8], AF.Square)
            last = len(_ORDER["ACT"]) - 1
            for pos, key in enumerate(_ORDER["ACT"]):
                _, k = key
                scalar.wait_ge(dmaA, 16 * min(k + 2, len(A_SPANS) + 1))
                b0, nb = A_SPANS[k]
                tile = abuf[k % NABUF]
                for b in range(nb):
                    inst = tag(scalar.activation(
                        junkA,
                        tile[:, b * BLK : (b + 1) * BLK],
                        AF.Square,
                        accum_out=bs[:, b0 + b : b0 + b + 1],
                    ), ("actA", k, b))
                inst.then_inc(aFree, 1)
                if pos == last:
                    scalar.drain().then_inc(actfin, 1)

        @block.vector
        def _(vector):
            vector.memset(junkA[:, 0:8], 0.0).then_inc(warm, 1)
            for pos, key in enumerate(_ORDER["DVE"]):
                kind, k = key
                if kind in ("ttr8", "ttr32"):
                    if kind == "ttr8":
                        vector.wait_ge(dmaT8, 16 * (k + 1))
                        b0, nb = T8_SPANS[k]
                        tile = t8buf[k % NT8BUF]
                        free_sem = t8Free
                    else:
                        vector.wait_ge(dmaT32, 16 * (k + 1))
                        b0, nb = T32_SPANS[k]
                        tile = t32buf[k % NT32BUF]
                        free_sem = t32Free
                    for b in range(nb):
                        slot = b0 + b - BN_BASE
                        for c in range(2):
                            inst = tag(vector.bn_stats(
                                stats[:, slot * 12 + c * 6 : slot * 12 + (c + 1) * 6],
                                tile[:, b * BLK + c * 480 : b * BLK + (c + 1) * 480],
                            ), (kind, k, b, c))
                    inst.then_inc(free_sem, 1)
                else:  # redP: pair-add 480->240 then segmented reduce
                    nP_ = len(P_SPANS)
                    w = 16 * (k + 1) if k < nP_ - 3 else 16 * min(k + 2, nP_ + 1)
                    vector.wait_ge(foldP, w)
                    b0, nb = P_SPANS[k]
                    v = (
                        sqbuf[k % NSQBUF][:, 0 : nb * BLK]
                        .rearrange("p (n b) -> p n b", b=BLK)
                    )
                    with nc.allow_low_precision("bf16 pair-add"):
                        tag(vector.tensor_tensor(
                            v[:, :, 480:720],
                            v[:, :, 0:240],
                            v[:, :, 240:480],
                            op=mybir.AluOpType.add,
                        ), ("addP", k))
                    inst = tag(vector.reduce_sum(
                        bs[:, b0 : b0 + nb], v[:, :, 480:720], axis=AX.X
                    ), ("redP", k))
                    inst.then_inc(sqFree, 1)
            vector.memset(junkA[:, 16:24], 0.0).then_inc(dvefin, 1)

    return nc


def make_in_maps(x_env: np.ndarray, y_env: np.ndarray) -> list[dict[str, np.ndarray]]:
    x = np.asarray(x_env, dtype=np.float32).reshape(ROWS, T)
    y = np.asarray(y_env, dtype=np.float32).reshape(ROWS, T)
    in_maps = []
    for i in range(N_CORES):
        shard = np.concatenate(
            [x[i * RPC : (i + 1) * RPC], y[i * RPC : (i + 1) * RPC]], axis=0
        )
        in_maps.append({"xy": np.ascontiguousarray(shard)})
    return in_maps


def assemble_bs(bs: np.ndarray, stats: np.ndarray) -> np.ndarray:
    """Fill the bn-route block sums (columns 84..156) from the device
    bn_stats output: per 480-chunk, sum(x^2) = M2_even + 240*mean_even^2 +
    M2_odd + 240*mean_odd^2; block = chunk0 + chunk1."""
    bs = np.array(bs, dtype=np.float64).reshape(128, NBLK)
    nbn = sum(T8_SIZES) + sum(T32_SIZES)
    st = np.asarray(stats, dtype=np.float64).reshape(128, nbn, 4, 3)
    ss = st[..., 2] + 240.0 * st[..., 1] ** 2  # [128, nbn, 4]
    b0 = A_SPANS[-1][0] + A_SPANS[-1][1]
    bs[:, b0 : b0 + nbn] = ss.sum(axis=2)
    return bs


def lufs_from_bs(bs: np.ndarray) -> np.ndarray:
    """Per-row gated lufs from [N, 200] block energy sums (float64 host math)."""
    bs = np.asarray(bs, dtype=np.float64)
    n = bs.shape[0]
    # frame f = blocks 3f..3f+9; cumulative sum gives all frame windows
    cs = np.concatenate([np.zeros((n, 1)), np.cumsum(bs, axis=1)], axis=1)
    starts = 3 * np.arange(NFRM)
    z = (cs[:, starts + 10] - cs[:, starts]) / FRAME  # [N, 64]
    el = -0.691 + 10.0 * np.log10(z + EPS)
    idx_a = (el > GAMMA_A).astype(np.float64)
    z_ave_a = (z * idx_a).sum(1, keepdims=True) / (idx_a.sum(1, keepdims=True) + EPS)
    gamma_r = -0.691 + 10.0 * np.log10(z_ave_a + EPS) - 10.0
    idx_ar = idx_a * (el > gamma_r)
    z_ave_ar = (z * idx_ar).sum(1, keepdims=True) / (idx_ar.sum(1, keepdims=True) + EPS)
    return (-0.691 + 10.0 * np.log10(z_ave_ar + EPS)).reshape(n)


def finish(per_core: list[tuple[np.ndarray, np.ndarray]]) -> np.ndarray:
    total = 0.0
    for bsc, stc in per_core:
        lufs = lufs_from_bs(assemble_bs(bsc, stc))
        total += np.maximum(lufs[RPC:] - lufs[:RPC], 0.0).sum()
    return np.array(ALPHA * total, dtype=np.float32)


def kernel(x_env: np.ndarray, y_env: np.ndarray) -> np.ndarray:
    nc = _build_program()
    in_maps = make_in_maps(x_env, y_env)
    res = run_bass_kernel_spmd(nc, in_maps, core_ids=list(range(N_CORES)))
    return finish(
        [(res.results[i]["bs"], res.results[i]["stats"]) for i in range(N_CORES)]
    )


# revision 49
# speedup vs baseline: 1.0337x; 1.0337x over previous
"""DHASPI level-loss kernel for 8 Trainium2 NeuronCores.

Data-parallel over the fused B*C row axis: each core gets 64 rows of x_env
(SBUF partitions 0-63) and 64 rows of y_env (partitions 64-127). The device
computes per-row energies of the 200 non-overlapping 960-sample blocks
(gcd(9600, 2880) — every overlapping loudness frame is a sum of 10 of
them); frame energies, gating, lufs, and the relu-diff loss are a tiny
float64 numpy epilogue on the host.

The 200 blocks per row are split across four routes chosen by a
linear-program balance of the four instruction queues (SP, Pool, ACT, DVE):

  route   load                 square + block reduction
  A  (84) Pool SWDGE cast->fp8 ACT Square with fused accum_out  (1172/blk)
  T8 (49) Pool SWDGE cast->fp8 DVE bn_stats per 480-chunk       (1120/blk)
  T32(24) SP HWDGE f32         DVE bn_stats per 480-chunk       (1120/blk)
  P  (43) SP HWDGE f32         Pool tensor_tensor square (bf16, 800/blk)
                               + Pool accum-DMA fold 960->480 (370/blk)
                               + DVE pair-add 480->240 + reduce_sum (385/blk)

Design notes:
- bn_stats (a real BIR opcode; the fused TENSOR_TENSOR_REDUCE ISA op does
  not codegen on this toolchain) computes per-480-chunk count/mean/M2 in
  f32; sum(x^2) = M2 + 240*mean_even^2 (+ odd half) is reassembled on the
  host from the "stats" output. Both ACT and bn_stats square in f32, so
  fp8 input costs only quantization noise whose squaring bias cancels
  between x and y.
- DMA completion semaphores fire per DMA-engine chunk, so a consumer that
  waits with zero margin can observe partition-banded pre-DMA data (the
  baseline was safe only via its scheduling slack). Defenses here: ACT
  waits one cast ahead (a tail dummy DMA supplies the final increments);
  the P-route fold is queued after the NEXT unit's square so the Q7 store
  pipe drains before the fold's DMA engines read it; the last P units
  wait one fold ahead (tail dummy fold); DVE-side consumers get a planned
  2us margin in the static schedule.
- Same-engine DVE RAW hazards (stale reads through the DVE write pipe):
  bn_stats reads only DMA-written tiles; the P-route reduce reads the
  pair-add output no faster than it was written.
- Engine instruction orders are static: consumer streams are built by a
  release-aware list scheduler iterated to a fixed point with producer
  timelines, and producers are deadline-sorted against consumer need
  times (margins included) with release clamping.
"""

import numpy as np

import concourse.bass as bass
from concourse import mybir
from concourse.bass_utils import run_bass_kernel_spmd

# Problem constants (hardcoded; kernel.py must be self-contained)
B, C, T = 16, 32, 192000
N_CORES = 8
ROWS = B * C  # 512
RPC = ROWS // N_CORES  # 64 rows per core per tensor

FRAME = 9600
SHIFT = 2880
BLK = 960  # gcd(FRAME, SHIFT)
NBLK = T // BLK  # 200 block sums per row
NFRM = (T - FRAME) // SHIFT + 1  # 64 frames per row

EPS = 1e-8
ALPHA = 1e-4
GAMMA_A = -70.0

F32 = mybir.dt.float32
BF16 = mybir.dt.bfloat16
FP8 = mybir.dt.float8e4

# Route unit sizes in 960-blocks (LP-balanced: A=83, T8=51, T32=26, P=40).
# Small first units cut pipeline fill.
A_SIZES = [1, 3] + [5] * 16  # 84 blocks -> ACT square+accum
T8_SIZES = [1, 3] + [5] * 9  # 49 blocks -> DVE bn_stats (fp8)
T32_SIZES = [2, 3, 4, 4, 4, 3, 3, 1]  # 24 blocks -> DVE bn_stats (f32)
P_SIZES = [1, 2] + [5] * 7 + [4, 1]  # 43 blocks -> Pool sq + fold + DVE add/red


def _spans(sizes, start):
    out = []
    for s in sizes:
        out.append((start, s))
        start += s
    return out, start


A_SPANS, _o = _spans(A_SIZES, 0)
T8_SPANS, _o = _spans(T8_SIZES, _o)
T32_SPANS, _o = _spans(T32_SIZES, _o)
P_SPANS, _o = _spans(P_SIZES, _o)
assert _o == NBLK

NABUF = 5  # fp8 ring for A route
NT8BUF = 5  # fp8 ring for T8 route
NT32BUF = 3  # f32 ring for T32 route
NPBUF = 4  # f32 ring for P route
NSQBUF = 3  # bf16 squared ring for P route

MAXU_A = max(A_SIZES) * BLK
MAXU_T8 = max(T8_SIZES) * BLK
MAXU_T32 = max(T32_SIZES) * BLK
MAXU_P = max(P_SIZES) * BLK

# ---------------------------------------------------------------------------
# Static scheduling: build one op list with dependencies and durations,
# list-schedule by latest-start-time, and use the per-engine order for
# emission.  Durations are V1-cost-model estimates in ns.

_DMA_LAT = 5500.0  # DMA init + completion-sem propagation + margin


def _dur(kind, nb):
    return {
        "cA": 370.6 * nb,
        "cT8": 370.6 * nb,
        "cT32": 1480.5 * nb,
        "cP": 1480.5 * nb,
        "sqP": 800.0 * nb,
        "f1P": max(370.6 * nb, 500.0),
        "actA": 1172.0 * nb,
        "ttr8": 1120.0 * nb,
        "ttr32": 1120.0 * nb,
        "redP": 400.0 * nb,
    }[kind]


def _schedule():
    """Three-pass schedule.

    1. Draft consumer streams (ACT sequential; DVE by target fractions),
       deadline-sort producers, and serially estimate SP/Pool timelines
       with release clamping.
    2. Rebuild the DVE stream by list-scheduling against estimated tile
       ready-times, so no DVE op sits in the stream before its data can
       exist.
    3. Re-sort producers against the final consumer need-times.
    """
    n8, n32, nP = len(T8_SPANS), len(T32_SPANS), len(P_SPANS)
    kind_spans = {"ttr8": T8_SPANS, "ttr32": T32_SPANS, "redP": P_SPANS}

    act_order = [("actA", k) for k in range(len(A_SPANS))]

    def act_need_of():
        need, t = {}, 0.0
        for kind, k in act_order:
            need[(kind, k)] = t
            t += _dur("actA", A_SPANS[k][1])
        return need

    act_need = act_need_of()

    def dve_need_of(order):
        need, t = {}, 0.0
        for kind, k in order:
            need[(kind, k)] = t
            t += _dur(kind, kind_spans[kind][k][1])
        return need, t

    # Pass 1 draft DVE order: fraction merge.
    units = []
    for k in range(n8):
        units.append(((k + 0.25) / n8, ("ttr8", k)))
    for k in range(n32):
        units.append(((k + 0.75) / n32, ("ttr32", k)))
    for k in range(nP):
        units.append(((k + 1.1) / (nP + 1.45), ("redP", k)))
    units.sort(key=lambda x: x[0])
    dve_order = [u for _, u in units]

    def producer_streams(dve_need, pool_done_prev=None):
        # consumer completion estimates (for ring-free releases)
        dve_done = {
            u: t + _dur(u[0], kind_spans[u[0]][u[1]][1])
            for u, t in dve_need.items()
        }
        act_done = {
            u: t + _dur("actA", A_SPANS[u[1]][1]) for u, t in act_need.items()
        }
        sem = 400.0  # engine-side sem propagation to the producer

        sp_ops = []
        for k in range(n32):
            rel = (
                dve_done[("ttr32", k - NT32BUF)] + sem if k >= NT32BUF else 0.0
            )
            dl = dve_need[("ttr32", k)] - _DMA_LAT
            sp_ops.append((dl, rel, ("cT32", k)))
        for k in range(nP):
            nb = P_SPANS[k][1]
            sq_dl = dve_need[("redP", k)] - _DMA_LAT - _dur("f1P", nb)
            rel = 0.0
            if k >= NPBUF and pool_done_prev is not None:
                rel = pool_done_prev.get(("sqP", k - NPBUF), 2900.0) + sem - 2600.0
            sp_ops.append(
                (sq_dl - _DMA_LAT - _dur("sqP", nb) - 6000.0, rel, ("cP", k))
            )
        sp_ops.sort(key=lambda x: x[0])
        t = 300.0
        sp_done = {}
        for _, rel, op in sp_ops:
            kind, k = op
            t = max(t, rel) + _dur(
                kind, (T32_SPANS if kind == "cT32" else P_SPANS)[k][1]
            )
            sp_done[op] = t + 2600.0
        pool_ops = []
        for k in range(len(A_SPANS)):
            rel = act_done[("actA", k - NABUF)] + sem if k >= NABUF else 0.0
            dl = act_need[("actA", max(k - 1, 0))] - _DMA_LAT
            pool_ops.append((dl, rel, ("cA", k)))
        for k in range(n8):
            rel = dve_done[("ttr8", k - NT8BUF)] + sem if k >= NT8BUF else 0.0
            pool_ops.append(
                (dve_need[("ttr8", k)] - _DMA_LAT - 1.0, rel, ("cT8", k))
            )
        sq_dls = {}
        for k in range(nP):
            nb = P_SPANS[k][1]
            f1_dl = dve_need[("redP", k)] - _DMA_LAT
            sq_dl = f1_dl - _dur("f1P", nb)
            rel = sp_done[("cP", k)]
            if k >= NSQBUF:
                rel = max(rel, dve_done[("redP", k - NSQBUF)] + sem)
            sq_dls[k] = (max(sq_dl, rel), rel, f1_dl)
            pool_ops.append((max(sq_dl, rel), rel, ("sqP", k)))
        for k in range(nP):
            _, rel, f1_dl = sq_dls[k]
            nxt = min(k + 1, nP - 1)
            after_next_sq = sq_dls[nxt][0] + _dur("sqP", P_SPANS[nxt][1]) + 1.0
            pool_ops.append(
                (max(f1_dl, rel + _dur("sqP", P_SPANS[k][1]), after_next_sq),
                 rel, ("f1P", k))
            )
        pool_ops.sort(key=lambda x: x[0])
        t = 300.0
        pool_done = {}
        for _, rel, op in pool_ops:
            kind, k = op
            nb = (
                A_SPANS[k][1]
                if kind == "cA"
                else T8_SPANS[k][1]
                if kind == "cT8"
                else P_SPANS[k][1]
            )
            t = max(t, rel) + _dur(kind, nb)
            pool_done[op] = t + 2600.0
        return (
            [op for _, _, op in sp_ops],
            [op for _, _, op in pool_ops],
            sp_done,
            pool_done,
        )

    dve_need, dve_total = dve_need_of(dve_order)

    # Passes 2..N: alternate (a) producer timelines from current DVE order
    # and (b) DVE re-list-scheduled against tile ready-times, to a fixed
    # point; then a final producer re-sort.
    pool_done = None
    for it in range(4):
        if it < 3:
            sp_order, pool_order, sp_done, pool_done = producer_streams(
                dve_need, pool_done
            )
        # else: final pass — freeze producers, re-derive only the DVE order
        ready = {}
        for k in range(n8):
            ready[("ttr8", k)] = pool_done[("cT8", k)] + 2000.0
        for k in range(n32):
            ready[("ttr32", k)] = sp_done[("cT32", k)] + 2000.0
        for k in range(nP):
            ready[("redP", k)] = (
                pool_done[("f1P", k)] + 2000.0
                if k < nP - 3
                else pool_done[("f1P", min(k + 1, nP - 1))] + 1000.0
            )
        pend = set(ready)
        # within a kind, units must stay in k order: only the lowest
        # unscheduled k of each kind is eligible.
        ptr = {"ttr8": 0, "ttr32": 0, "redP": 0}
        now = 0.0
        new_order = []
        while pend:
            elig = [
                (kind, ptr[kind])
                for kind in ptr
                if (kind, ptr[kind]) in pend
            ]
            ready_now = [u for u in elig if ready[u] <= now]
            pick = (
                min(ready_now, key=lambda u: ready[u])
                if ready_now
                else min(elig, key=lambda u: ready[u])
            )
            now = max(now, ready[pick]) + _dur(
                pick[0], kind_spans[pick[0]][pick[1]][1]
            )
            new_order.append(pick)
            ptr[pick[0]] += 1
            pend.remove(pick)
        dve_order = new_order
        dve_need, dve_total = dve_need_of(dve_order)

    # f1P_k's emission waits on sqP_{min(k+1, nP-1)} (drain fence): enforce
    # that queue order or the Pool queue deadlocks on its own wait.
    reordered = []
    pending = {}  # required sq index -> [fold ops]
    seen_sq = -1
    for op in pool_order:
        kind, k = op
        if kind == "f1P":
            need = min(k + 1, nP - 1)
            if need > seen_sq:
                pending.setdefault(need, []).append(op)
                continue
            reordered.append(op)
        else:
            reordered.append(op)
            if kind == "sqP":
                seen_sq = k
                for j in sorted(list(pending)):
                    if j <= seen_sq:
                        reordered.extend(pending.pop(j))
    for j in sorted(list(pending)):
        reordered.extend(pending.pop(j))
    pool_order = reordered

    order = {
        "SP": sp_order,
        "Pool": pool_order,
        "ACT": act_order,
        "DVE": dve_order,
    }
    for eng, seq in order.items():
        by_kind = {}
        for i, (kind, k) in enumerate(seq):
            by_kind.setdefault(kind, []).append((i, k))
        for kind, pairs in by_kind.items():
            ks = sorted(k for _, k in pairs)
            for (i, _), k in zip(pairs, ks):
                seq[i] = (kind, k)
    return order, dve_total


_ORDER, _EST_FINISH = _schedule()


def _build_program() -> bass.Bass:
    nc = bass.Bass("TRN2", target_bir_lowering=False, debug=False)
    nc._op_labels = {}

    def tag(inst, label):
        nc._op_labels[inst.ins.name] = label
        return inst

    AF = mybir.ActivationFunctionType
    ALU = mybir.AluOpType
    AX = mybir.AxisListType

    xy = nc.dram_tensor("xy", [128, T], F32, kind="ExternalInput").ap()
    out = nc.dram_tensor("bs", [128, NBLK], F32, kind="ExternalOutput").ap()
    out_st = nc.dram_tensor(
        "stats", [128, (sum(T8_SIZES) + sum(T32_SIZES)) * 12], F32,
        kind="ExternalOutput",
    ).ap()

    abuf = [
        nc.alloc_sbuf_tensor(f"ab{i}", [128, MAXU_A], FP8).ap() for i in range(NABUF)
    ]
    t8buf = [
        nc.alloc_sbuf_tensor(f"t8b{i}", [128, MAXU_T8], FP8).ap()
        for i in range(NT8BUF)
    ]
    t32buf = [
        nc.alloc_sbuf_tensor(f"t32b{i}", [128, MAXU_T32], F32).ap()
        for i in range(NT32BUF)
    ]
    pbuf = [
        nc.alloc_sbuf_tensor(f"pb{i}", [128, MAXU_P], F32).ap() for i in range(NPBUF)
    ]
    sqbuf = [
        nc.alloc_sbuf_tensor(f"sq{i}", [128, MAXU_P], BF16).ap()
        for i in range(NSQBUF)
    ]
    bs = nc.alloc_sbuf_tensor("bst", [128, NBLK], F32).ap()
    junkA = nc.alloc_sbuf_tensor("junkA", [128, BLK], BF16).ap()
    nbn = sum(T8_SIZES) + sum(T32_SIZES)  # 73 bn blocks
    stats = nc.alloc_sbuf_tensor("stats_sb", [128, nbn * 12], F32).ap()
    BN_BASE = A_SPANS[-1][0] + A_SPANS[-1][1]  # first bn block index (84)

    with (
        nc.Block() as block,
        nc.semaphore("dmaA") as dmaA,
        nc.semaphore("dmaT8") as dmaT8,
        nc.semaphore("dmaT32") as dmaT32,
        nc.semaphore("dmaP") as dmaP,
        nc.semaphore("foldP") as foldP,
        nc.semaphore("aFree") as aFree,
        nc.semaphore("t8Free") as t8Free,
        nc.semaphore("t32Free") as t32Free,
        nc.semaphore("pf32Free") as pf32Free,
        nc.semaphore("sqFree") as sqFree,
        nc.semaphore("actfin") as actfin,
        nc.semaphore("dvefin") as dvefin,
        nc.semaphore("warm") as warm,
        nc.semaphore("bnfin") as bnfin,
        nc.semaphore("outs") as outs,
    ):
        @block.sync
        def _(sync):
            for key in _ORDER["SP"]:
                kind, k = key
                if kind == "cT32":
                    if k >= NT32BUF:
                        sync.wait_ge(t32Free, k - NT32BUF + 1)
                    b0, nb = T32_SPANS[k]
                    tag(sync.dma_start(
                        out=t32buf[k % NT32BUF][:, 0 : nb * BLK],
                        in_=xy[:, b0 * BLK : (b0 + nb) * BLK],
                    ), ("cT32", k)).then_inc(dmaT32, 16)
                else:  # cP
                    if k >= NPBUF:
                        sync.wait_ge(pf32Free, k - NPBUF + 1)
                    b0, nb = P_SPANS[k]
                    tag(sync.dma_start(
                        out=pbuf[k % NPBUF][:, 0 : nb * BLK],
                        in_=xy[:, b0 * BLK : (b0 + nb) * BLK],
                    ), ("cP", k)).then_inc(dmaP, 16)
            nA = A_SPANS[-1][0] + A_SPANS[-1][1]
            p0 = P_SPANS[0][0]
            sync.wait_ge(actfin, 1)
            sync.dma_start(out=out[:, 0:nA], in_=bs[:, 0:nA]).then_inc(outs, 16)
            sync.wait_ge(dvefin, 1)
            sync.dma_start(
                out=out[:, p0:NBLK], in_=bs[:, p0:NBLK]
            ).then_inc(outs, 16)
            sync.wait_ge(outs, 48)

        @block.gpsimd
        def _(g):
            n_cA = len(A_SPANS) - 1
            n_f1 = len(P_SPANS) - 1
            for key in _ORDER["Pool"]:
                kind, k = key
                if kind == "cA":
                    if k >= NABUF:
                        g.wait_ge(aFree, k - NABUF + 1)
                    b0, nb = A_SPANS[k]
                    tag(g.dma_start(
                        out=abuf[k % NABUF][:, 0 : nb * BLK],
                        in_=xy[:, b0 * BLK : (b0 + nb) * BLK],
                    ), ("cA", k)).then_inc(dmaA, 16)
                    if k == n_cA:
                        # tail dummy: its 16 chunk-completions certify every
                        # DMA engine has drained the last real cA transfer
                        g.dma_start(
                            out=junkA[:, 128:256], in_=xy[:, 0:128]
                        ).then_inc(dmaA, 16)
                elif kind == "cT8":
                    if k >= NT8BUF:
                        g.wait_ge(t8Free, k - NT8BUF + 1)
                    b0, nb = T8_SPANS[k]
                    tag(g.dma_start(
                        out=t8buf[k % NT8BUF][:, 0 : nb * BLK],
                        in_=xy[:, b0 * BLK : (b0 + nb) * BLK],
                    ), ("cT8", k)).then_inc(dmaT8, 16)
                elif kind == "sqP":
                    g.wait_ge(dmaP, 16 * (k + 1))
                    if k >= NSQBUF:
                        g.wait_ge(sqFree, k - NSQBUF + 1)
                    b0, nb = P_SPANS[k]
                    tag(g.tensor_tensor(
                        sqbuf[k % NSQBUF][:, 0 : nb * BLK],
                        pbuf[k % NPBUF][:, 0 : nb * BLK],
                        pbuf[k % NPBUF][:, 0 : nb * BLK],
                        op=ALU.mult,
                    ), ("sqP", k)).then_inc(pf32Free, 1)
                else:  # f1P: fold 960 -> 480, in place, accum add
                    g.wait_ge(pf32Free, min(k + 2, len(P_SPANS)))
                    b0, nb = P_SPANS[k]
                    v = (
                        sqbuf[k % NSQBUF][:, 0 : nb * BLK]
                        .rearrange("p (n b) -> p n b", b=BLK)
                    )
                    tag(g.dma_start(
                        out=v[:, :, 0:480], in_=v[:, :, 480:960], accum_op=ALU.add
                    ), ("f1P", k)).then_inc(foldP, 16)
                    if k == n_f1:
                        g.dma_start(
                            out=junkA[:, 256:320], in_=junkA[:, 320:384],
                            accum_op=ALU.add,
                        ).then_inc(foldP, 16)

        @block.scalar
        def _(scalar):
            # Warm the Square activation table during pipeline fill so the
            # 1383ns table load is off the first real unit's critical path.
            # (Input is a DVE-zeroed scratch; the output is never consumed.)
            scalar.wait_ge(warm, 1)
            scalar.activation(junkA[:, 8:16], junkA[:, 0:8], AF.Square)
            last = len(_ORDER["ACT"]) - 1
            for pos, key in enumerate(_ORDER["ACT"]):
                _, k = key
                scalar.wait_ge(dmaA, 16 * min(k + 2, len(A_SPANS) + 1))
                b0, nb = A_SPANS[k]
                tile = abuf[k % NABUF]
                for b in range(nb):
                    inst = tag(scalar.activation(
                        junkA,
                        tile[:, b * BLK : (b + 1) * BLK],
                        AF.Square,
                        accum_out=bs[:, b0 + b : b0 + b + 1],
                    ), ("actA", k, b))
                inst.then_inc(aFree, 1)
                if pos == last:
                    scalar.drain().then_inc(actfin, 1)
            scalar.wait_ge(bnfin, 1)
            scalar.dma_start(out=out_st, in_=stats).then_inc(outs, 16)

        @block.vector
        def _(vector):
            vector.memset(junkA[:, 0:8], 0.0).then_inc(warm, 1)
            bn_last = max(
                i for i, (kk, _) in enumerate(_ORDER["DVE"]) if kk.startswith("ttr")
            )
            for pos, key in enumerate(_ORDER["DVE"]):
                kind, k = key
                if kind in ("ttr8", "ttr32"):
                    if kind == "ttr8":
                        vector.wait_ge(dmaT8, 16 * (k + 1))
                        b0, nb = T8_SPANS[k]
                        tile = t8buf[k % NT8BUF]
                        free_sem = t8Free
                    else:
                        vector.wait_ge(dmaT32, 16 * (k + 1))
                        b0, nb = T32_SPANS[k]
                        tile = t32buf[k % NT32BUF]
                        free_sem = t32Free
                    for b in range(nb):
                        slot = b0 + b - BN_BASE
                        for c in range(2):
                            inst = tag(vector.bn_stats(
                                stats[:, slot * 12 + c * 6 : slot * 12 + (c + 1) * 6],
                                tile[:, b * BLK + c * 480 : b * BLK + (c + 1) * 480],
                            ), (kind, k, b, c))
                    inst.then_inc(free_sem, 1)
                else:  # redP: pair-add 480->240 then segmented reduce
                    nP_ = len(P_SPANS)
                    w = 16 * (k + 1) if k < nP_ - 3 else 16 * min(k + 2, nP_ + 1)
                    vector.wait_ge(foldP, w)
                    b0, nb = P_SPANS[k]
                    v = (
                        sqbuf[k % NSQBUF][:, 0 : nb * BLK]
                        .rearrange("p (n b) -> p n b", b=BLK)
                    )
                    with nc.allow_low_precision("bf16 pair-add"):
                        tag(vector.tensor_tensor(
                            v[:, :, 480:720],
                            v[:, :, 0:240],
                            v[:, :, 240:480],
                            op=mybir.AluOpType.add,
                        ), ("addP", k))
                    inst = tag(vector.reduce_sum(
                        bs[:, b0 : b0 + nb], v[:, :, 480:720], axis=AX.X
                    ), ("redP", k))
                    inst.then_inc(sqFree, 1)
                if pos == bn_last:
                    vector.memset(junkA[:, 24:32], 0.0).then_inc(bnfin, 1)
            vector.memset(junkA[:, 16:24], 0.0).then_inc(dvefin, 1)

    return nc


def make_in_maps(x_env: np.ndarray, y_env: np.ndarray) -> list[dict[str, np.ndarray]]:
    x = np.asarray(x_env, dtype=np.float32).reshape(ROWS, T)
    y = np.asarray(y_env, dtype=np.float32).reshape(ROWS, T)
    in_maps = []
    for i in range(N_CORES):
        shard = np.concatenate(
            [x[i * RPC : (i + 1) * RPC], y[i * RPC : (i + 1) * RPC]], axis=0
        )
        in_maps.append({"xy": np.ascontiguousarray(shard)})
    return in_maps


def assemble_bs(bs: np.ndarray, stats: np.ndarray) -> np.ndarray:
    """Fill the bn-route block sums (columns 84..156) from the device
    bn_stats output: per 480-chunk, sum(x^2) = M2_even + 240*mean_even^2 +
    M2_odd + 240*mean_odd^2; block = chunk0 + chunk1."""
    bs = np.array(bs, dtype=np.float64).reshape(128, NBLK)
    nbn = sum(T8_SIZES) + sum(T32_SIZES)
    st = np.asarray(stats, dtype=np.float64).reshape(128, nbn, 4, 3)
    ss = st[..., 2] + 240.0 * st[..., 1] ** 2  # [128, nbn, 4]
    b0 = A_SPANS[-1][0] + A_SPANS[-1][1]
    bs[:, b0 : b0 + nbn] = ss.sum(axis=2)
    return bs


def lufs_from_bs(bs: np.ndarray) -> np.ndarray:
    """Per-row gated lufs from [N, 200] block energy sums (float64 host math)."""
    bs = np.asarray(bs, dtype=np.float64)
    n = bs.shape[0]
    # frame f = blocks 3f..3f+9; cumulative sum gives all frame windows
    cs = np.concatenate([np.zeros((n, 1)), np.cumsum(bs, axis=1)], axis=1)
    starts = 3 * np.arange(NFRM)
    z = (cs[:, starts + 10] - cs[:, starts]) / FRAME  # [N, 64]
    el = -0.691 + 10.0 * np.log10(z + EPS)
    idx_a = (el > GAMMA_A).astype(np.float64)
    z_ave_a = (z * idx_a).sum(1, keepdims=True) / (idx_a.sum(1, keepdims=True) + EPS)
    gamma_r = -0.691 + 10.0 * np.log10(z_ave_a + EPS) - 10.0
    idx_ar = idx_a * (el > gamma_r)
    z_ave_ar = (z * idx_ar).sum(1, keepdims=True) / (idx_ar.sum(1, keepdims=True) + EPS)
    return (-0.691 + 10.0 * np.log10(z_ave_ar + EPS)).reshape(n)


def finish(per_core: list[tuple[np.ndarray, np.ndarray]]) -> np.ndarray:
    total = 0.0
    for bsc, stc in per_core:
        lufs = lufs_from_bs(assemble_bs(bsc, stc))
        total += np.maximum(lufs[RPC:] - lufs[:RPC], 0.0).sum()
    return np.array(ALPHA * total, dtype=np.float32)


def kernel(x_env: np.ndarray, y_env: np.ndarray) -> np.ndarray:
    nc = _build_program()
    in_maps = make_in_maps(x_env, y_env)
    res = run_bass_kernel_spmd(nc, in_maps, core_ids=list(range(N_CORES)))
    return finish(
        [(res.results[i]["bs"], res.results[i]["stats"]) for i in range(N_CORES)]
    )
